# revision 18
# baseline (speedup 1.0000x reference)
"""Trainium2 Bass kernel for nn_Lookback: causal running-mean over T.

out[b, t, c] = (1/(t+1)) * sum_{s<=t} x[b, s, c],  x: [8, 4096, 1024] fp32.

Sharding: data-parallel over batch B — core b handles x[b] ([4096, 1024]).

The rel-err gate is 2e-2, so all device IO is bf16 (error ~3e-3): input is
downcast on the host, output upcast on the host. That halves HBM traffic.

Per-core algorithm (T split into 43 tiles: 42 tiles of 96 rows + one of 64).
Per tile k and column half h (512 cols), a 2-matmul PSUM accumulation group:
  main-mm:  W1[0:96]^T @ x_k  — causal cumsum of the tile's own rows; output
            partitions 96..127 get the full tile sum (triu extends there)
  carry-mm: ones[32,128]^T @ os_[96:128, col k-1] — adds S_{k-1}, the global
            running sum, read back from the PREVIOUS tile's eviction
The single eviction per half (alternating DVE for h0 / ACT for h1) scales
rows 0..95 by 1/(t+1) (the outputs) and rows 96..127 by 1/32 (32 identical
copies of S_k that become the next tile's carry operand) — so the carry
hand-off costs no extra DVE/ACT op and no PSUM->SBUF extract.
Two independent serial chains (one per column half, one per engine):
  ev_h(k) -> carry-mm_h(k+1) -> ev_h(k+1), ~1.1us per tile each.

DMA: batched loads (6x 7-tile groups + tail) then batched stores, all on the
sync HWDGE queue (stores are emitted after all loads, so a store waiting on
evictions never blocks a load).
"""

import sys

import numpy as np

sys.path.insert(0, "/opt/trn_rl_repo")

import ml_dtypes

import concourse.bass as bass
import concourse.mybir as mybir
import concourse.tile as tile
from concourse import bacc
from concourse.bass_utils import run_bass_kernel_spmd

B, T, C = 8, 4096, 1024
P = 96               # x rows per tile
NT = 43              # 42 full tiles + one 64-row tail tile
LAST = T - 42 * P    # 64
STORE_GROUPS = [7, 7, 7, 7, 5, 4, 3, 2]  # de-ramped so the tail drains fast
LOAD_GROUPS = [1, 1, 2, 3, 7, 7, 7, 7, 7]  # ramped so early tiles arrive ASAP
H = 512              # matmul N chunk (max moving free dim)
BF16 = mybir.dt.bfloat16
F32 = mybir.dt.float32
NFILL = 8            # filler matmuls per tile: keep the HAM clock at 8/8
NWARM = 44           # warm-up matmuls: flip the HAM clock before the loop

_cache = {}


def _consts():
    # W1[q, p] = [q <= p]: causal cumsum weights; for p >= 96 every q < 96
    # qualifies, so output rows 96..127 hold the full tile column-sum.
    w1 = np.triu(np.ones((128, 128), np.float32))
    # carry weights: rows 96..127 all-ones (os_ carry rows hold S/32);
    # full 128-partition tensor so the lhsT slice starts at partition 96,
    # matching the rhs start (walrus requires equal fmap/weight starts)
    wc = np.zeros((128, 128), np.float32)
    wc[96:, :] = 1.0
    # evict scale: rows 0..95 -> 1/(96k+p+1); rows 96..127 -> 1/32
    t_idx = (
        np.arange(128, dtype=np.float64)[:, None]
        + P * np.arange(NT, dtype=np.float64)[None, :]
    )
    recip = (1.0 / (t_idx + 1.0)).astype(np.float32)
    recip[P:, :] = 1.0 / 32.0
    # dense all-ones (rows 0..95): adds sum(x of the pair's even tile)
    wd = np.zeros((128, 128), np.float32)
    wd[:96, :] = 1.0
    wb = np.concatenate([w1, wc, wd], axis=1)
    return wb.astype(ml_dtypes.bfloat16), recip


def _build():
    nc = bacc.Bacc("TRN2", target_bir_lowering=False, debug=False, num_devices=B)
    x_d = nc.dram_tensor("x", [T, C], BF16, kind="ExternalInput").ap()
    wb_d = nc.dram_tensor("wb", [128, 384], BF16, kind="ExternalInput").ap()
    r_d = nc.dram_tensor("recip", [128, NT], F32, kind="ExternalInput").ap()
    out_d = nc.dram_tensor("out", [T, C], BF16, kind="ExternalOutput").ap()

    with tile.TileContext(nc) as tc:
        with (
            tc.tile_pool(name="const", bufs=1) as cp,
            tc.tile_pool(name="xbuf", bufs=1) as xp,
            tc.tile_pool(name="obuf", bufs=1) as obp,
            tc.tile_pool(name="ps", bufs=3, space=bass.MemorySpace.PSUM) as psp,
            tc.tile_pool(name="dmy", bufs=1, space=bass.MemorySpace.PSUM) as dpp,
        ):
            wb_s = cp.tile([128, 384], BF16)
            r_s = cp.tile([128, NT], F32)
            nc.sync.dma_start(wb_s[:], wb_d)
            nc.sync.dma_start(r_s[:], r_d)
            w1_s = wb_s[:, 0:128]
            wc_s = wb_s[:, 128:256]
            wd_s = wb_s[:, 256:384]

            xs = xp.tile([128, NT * C], BF16)    # x tiles (rows 0..95)
            os_ = obp.tile([128, NT * C], BF16)  # outputs + carry rows 96..127

            # ramped batched loads + the 64-row tail
            t0 = 0
            for gn in LOAD_GROUPS:
                r0 = t0 * P
                src = x_d[r0 : r0 + gn * P, :].rearrange("(n p) c -> p n c", p=P)
                dst = xs[0:P, t0 * C : (t0 + gn) * C].rearrange(
                    "p (n c) -> p n c", c=C
                )
                nc.sync.dma_start(dst, src)
                t0 += gn
            nc.sync.dma_start(xs[0:LAST, 42 * C : 43 * C], x_d[42 * P : T, :])

            # PE warm-up: a long unbroken burst of identical matmuls is what
            # flips the HAM clock gate to 8/8 (~4.2us sustained). Scratch is
            # memset (not DMA-loaded) so the burst starts at sequencer boot.
            dmy = dpp.tile([128, 128], F32)
            scr = cp.tile([128, 128], BF16)
            nc.gpsimd.memset(scr[:], 0.02)
            for _ in range(NWARM):
                nc.tensor.matmul(dmy[:], scr[:], scr[:], start=True, stop=True)
            store_bounds = []
            acc = 0
            for gn in STORE_GROUPS:
                acc += gn
                store_bounds.append(acc)
            for k in range(NT):
                rows = P if k < NT - 1 else LAST
                ck = k * C
                ps0 = psp.tile([128, H], F32, tag="ps0")
                ps1 = psp.tile([128, H], F32, tag="ps1")
                psh = [ps0, ps1]
                for h in range(2):
                    nc.tensor.matmul(
                        psh[h][:],
                        w1_s[0:rows, 0:128],
                        xs[0:rows, ck + h * H : ck + (h + 1) * H],
                        start=True,
                        stop=(k == 0),
                    )
                    if k > 0:
                        # += S_{k-1}: carry rows of the previous eviction
                        nc.tensor.matmul(
                            psh[h][:],
                            wc_s[96:128, :],
                            os_[P:128, ck - C + h * H : ck - C + (h + 1) * H],
                            start=False,
                            stop=True,
                            tile_position=(96, 0),
                        )
                # evict: rows 0..95 scaled outputs, rows 96..127 = S_k/32.
                # Each 512-col half is split across DVE and ACT so the
                # carry-mm's wait is ~470ns instead of ~750ns.
                scale = r_s[:, k : k + 1]
                Q = H // 2
                for h in range(2):
                    base = ck + h * H
                    nc.vector.tensor_scalar_mul(
                        os_[:, base : base + Q], psh[h][:, 0:Q], scale
                    )
                    nc.scalar.activation(
                        os_[:, base + Q : base + H], psh[h][:, Q:H],
                        mybir.ActivationFunctionType.Copy, scale=scale,
                    )
                if k >= 1:
                    for _ in range(NFILL):
                        nc.tensor.matmul(
                            dmy[:], w1_s[0:P, 0:128],
                            os_[0:P, ck - C : ck - C + 128],
                            start=True, stop=True,
                        )
                if k + 1 in store_bounds:
                    s0 = store_bounds[store_bounds.index(k + 1) - 1] if store_bounds.index(k + 1) > 0 else 0
                    r0 = s0 * P
                    gn = k + 1 - s0
                    dst = out_d[r0 : r0 + gn * P, :].rearrange(
                        "(n p) c -> p n c", p=P
                    )
                    src = os_[0:P, s0 * C : (k + 1) * C].rearrange(
                        "p (n c) -> p n c", c=C
                    )
                    nc.sync.dma_start(dst, src)
            nc.sync.dma_start(out_d[42 * P : T, :], os_[0:LAST, 42 * C : 43 * C])

    nc.compile()
    return nc


def _run(x, trace=False):
    x = np.ascontiguousarray(x, dtype=np.float32)
    assert x.shape == (B, T, C)
    if "nc" not in _cache:
        _cache["nc"] = _build()
        _cache["consts"] = _consts()
    nc = _cache["nc"]
    wb, recip = _cache["consts"]
    xb = x.astype(ml_dtypes.bfloat16)
    in_maps = [{"x": xb[b], "wb": wb, "recip": recip} for b in range(B)]
    res = run_bass_kernel_spmd(nc, in_maps, core_ids=list(range(B)), trace=trace)
    out = np.stack(
        [np.asarray(res.results[b]["out"]).astype(np.float32) for b in range(B)]
    )
    return out, res


def kernel(x):
    out, _ = _run(x, trace=False)
    return out


# revision 19
# speedup vs baseline: 1.9621x; 1.9621x over previous
"""Trainium2 Bass kernel for nn_Lookback: causal running-mean over T.

out[b, t, c] = (1/(t+1)) * sum_{s<=t} x[b, s, c],  x: [8, 4096, 1024] fp32.

Sharding: data-parallel over batch B — core b handles x[b] ([4096, 1024]).

The rel-err gate is 2e-2, so all device IO is bf16 (error ~3e-3): input is
downcast on the host, output upcast on the host. That halves HBM traffic.

Per-core algorithm (T split into 43 tiles: 42 tiles of 96 rows + one of 64).
Per tile k and column half h (512 cols), a 2-matmul PSUM accumulation group:
  main-mm:  W1[0:96]^T @ x_k  — causal cumsum of the tile's own rows; output
            partitions 96..127 get the full tile sum (triu extends there)
  carry-mm: ones[32,128]^T @ os_[96:128, col k-1] — adds S_{k-1}, the global
            running sum, read back from the PREVIOUS tile's eviction
The single eviction per half (alternating DVE for h0 / ACT for h1) scales
rows 0..95 by 1/(t+1) (the outputs) and rows 96..127 by 1/32 (32 identical
copies of S_k that become the next tile's carry operand) — so the carry
hand-off costs no extra DVE/ACT op and no PSUM->SBUF extract.
Two independent serial chains (one per column half, one per engine):
  ev_h(k) -> carry-mm_h(k+1) -> ev_h(k+1), ~1.1us per tile each.

DMA: batched loads (6x 7-tile groups + tail) then batched stores, all on the
sync HWDGE queue (stores are emitted after all loads, so a store waiting on
evictions never blocks a load).
"""

import sys

import numpy as np

sys.path.insert(0, "/opt/trn_rl_repo")

import ml_dtypes

import concourse.bass as bass
import concourse.mybir as mybir
import concourse.tile as tile
from concourse import bacc
from concourse.bass_utils import run_bass_kernel_spmd

B, T, C = 8, 4096, 1024
P = 96               # x rows per tile
NT = 43              # 42 full tiles + one 64-row tail tile
LAST = T - 42 * P    # 64
STORE_GROUPS = [7, 7, 7, 7, 5, 4, 3, 2]  # de-ramped so the tail drains fast
LOAD_GROUPS = [1, 1, 2, 3, 7, 7, 7, 7, 7]  # ramped so early tiles arrive ASAP
H = 512              # matmul N chunk (max moving free dim)
BF16 = mybir.dt.bfloat16
F32 = mybir.dt.float32
NFILL = 8            # filler matmuls per tile: keep the HAM clock at 8/8
NWARM = 44           # warm-up matmuls: flip the HAM clock before the loop

_cache = {}


def _consts():
    # W1[q, p] = [q <= p]: causal cumsum weights; for p >= 96 every q < 96
    # qualifies, so output rows 96..127 hold the full tile column-sum.
    w1 = np.triu(np.ones((128, 128), np.float32))
    # carry weights: rows 96..127 all-ones (os_ carry rows hold S/32);
    # full 128-partition tensor so the lhsT slice starts at partition 96,
    # matching the rhs start (walrus requires equal fmap/weight starts)
    wc = np.zeros((128, 128), np.float32)
    wc[96:, :] = 1.0
    # evict scale: rows 0..95 -> 1/(96k+p+1); rows 96..127 -> 1/32
    t_idx = (
        np.arange(128, dtype=np.float64)[:, None]
        + P * np.arange(NT, dtype=np.float64)[None, :]
    )
    recip = (1.0 / (t_idx + 1.0)).astype(np.float32)
    recip[P:, :] = 1.0 / 32.0
    # dense all-ones (rows 0..95): adds sum(x of the pair's even tile)
    wd = np.zeros((128, 128), np.float32)
    wd[:96, :] = 1.0
    wb = np.concatenate([w1, wc, wd], axis=1)
    return wb.astype(ml_dtypes.bfloat16), recip


def _build():
    nc = bacc.Bacc("TRN2", target_bir_lowering=False, debug=False, num_devices=B)
    x_d = nc.dram_tensor("x", [T, C], BF16, kind="ExternalInput").ap()
    wb_d = nc.dram_tensor("wb", [128, 384], BF16, kind="ExternalInput").ap()
    r_d = nc.dram_tensor("recip", [128, NT], F32, kind="ExternalInput").ap()
    out_d = nc.dram_tensor("out", [T, C], BF16, kind="ExternalOutput").ap()

    with tile.TileContext(nc) as tc:
        with (
            tc.tile_pool(name="const", bufs=1) as cp,
            tc.tile_pool(name="xbuf", bufs=1) as xp,
            tc.tile_pool(name="obuf", bufs=1) as obp,
            tc.tile_pool(name="ps", bufs=3, space=bass.MemorySpace.PSUM) as psp,
            tc.tile_pool(name="dmy", bufs=1, space=bass.MemorySpace.PSUM) as dpp,
        ):
            wb_s = cp.tile([128, 384], BF16)
            r_s = cp.tile([128, NT], F32)
            nc.sync.dma_start(wb_s[:], wb_d)
            nc.sync.dma_start(r_s[:], r_d)
            w1_s = wb_s[:, 0:128]
            wc_s = wb_s[:, 128:256]
            wd_s = wb_s[:, 256:384]

            xs = xp.tile([128, NT * C], BF16)    # x tiles (rows 0..95)
            os_ = obp.tile([128, NT * C], BF16)  # outputs + carry rows 96..127

            # ramped batched loads + the 64-row tail
            t0 = 0
            for gn in LOAD_GROUPS:
                r0 = t0 * P
                src = x_d[r0 : r0 + gn * P, :].rearrange("(n p) c -> p n c", p=P)
                dst = xs[0:P, t0 * C : (t0 + gn) * C].rearrange(
                    "p (n c) -> p n c", c=C
                )
                nc.sync.dma_start(dst, src)
                t0 += gn
            nc.sync.dma_start(xs[0:LAST, 42 * C : 43 * C], x_d[42 * P : T, :])

            # PE warm-up: a long unbroken burst of identical matmuls is what
            # flips the HAM clock gate to 8/8 (~4.2us sustained). Scratch is
            # memset (not DMA-loaded) so the burst starts at sequencer boot.
            dmy = dpp.tile([128, 128], F32)
            scr = cp.tile([128, 128], BF16)
            nc.gpsimd.memset(scr[:], 0.02)
            for _ in range(NWARM):
                nc.tensor.matmul(dmy[:], scr[:], scr[:], start=True, stop=True)
            store_bounds = []
            acc = 0
            for gn in STORE_GROUPS:
                acc += gn
                store_bounds.append(acc)
            for k in range(NT):
                rows = P if k < NT - 1 else LAST
                ck = k * C
                ps0 = psp.tile([128, H], F32, tag="ps0")
                ps1 = psp.tile([128, H], F32, tag="ps1")
                psh = [ps0, ps1]
                for h in range(2):
                    nc.tensor.matmul(
                        psh[h][:],
                        w1_s[0:rows, 0:128],
                        xs[0:rows, ck + h * H : ck + (h + 1) * H],
                        start=True,
                        stop=(k == 0),
                    )
                    if k > 0:
                        # += S_{k-1}: carry rows of the previous eviction
                        nc.tensor.matmul(
                            psh[h][:],
                            wc_s[96:128, :],
                            os_[P:128, ck - C + h * H : ck - C + (h + 1) * H],
                            start=False,
                            stop=True,
                            tile_position=(96, 0),
                        )
                # evict: rows 0..95 scaled outputs, rows 96..127 = S_k/32
                scale = r_s[:, k : k + 1]
                nc.vector.tensor_scalar_mul(
                    os_[:, ck : ck + H], psh[0][:], scale
                )
                nc.scalar.activation(
                    os_[:, ck + H : ck + C], psh[1][:],
                    mybir.ActivationFunctionType.Copy, scale=scale,
                )
                if k >= 1:
                    for _ in range(NFILL):
                        nc.tensor.matmul(
                            dmy[:], w1_s[0:P, 0:128],
                            os_[0:P, ck - C : ck - C + 128],
                            start=True, stop=True,
                        )
                if k + 1 in store_bounds:
                    s0 = store_bounds[store_bounds.index(k + 1) - 1] if store_bounds.index(k + 1) > 0 else 0
                    r0 = s0 * P
                    gn = k + 1 - s0
                    dst = out_d[r0 : r0 + gn * P, :].rearrange(
                        "(n p) c -> p n c", p=P
                    )
                    src = os_[0:P, s0 * C : (k + 1) * C].rearrange(
                        "p (n c) -> p n c", c=C
                    )
                    nc.sync.dma_start(dst, src)
            nc.sync.dma_start(out_d[42 * P : T, :], os_[0:LAST, 42 * C : 43 * C])

    nc.compile()
    return nc


def _run(x, trace=False):
    x = np.ascontiguousarray(x, dtype=np.float32)
    assert x.shape == (B, T, C)
    if "nc" not in _cache:
        _cache["nc"] = _build()
        _cache["consts"] = _consts()
    nc = _cache["nc"]
    wb, recip = _cache["consts"]
    xb = x.astype(ml_dtypes.bfloat16)
    in_maps = [{"x": xb[b], "wb": wb, "recip": recip} for b in range(B)]
    res = run_bass_kernel_spmd(nc, in_maps, core_ids=list(range(B)), trace=trace)
    out = np.stack(
        [np.asarray(res.results[b]["out"]).astype(np.float32) for b in range(B)]
    )
    return out, res


def kernel(x):
    out, _ = _run(x, trace=False)
    return out
